# revision 1
# baseline (speedup 1.0000x reference)
"""Two-layer GAT (DGL GATConv) on 8 Trainium2 NeuronCores.

Strategy (edge-parallel, dst-sorted):
  * Host sorts edges by dst; core c owns dst nodes [6250c, 6250(c+1)).
  * Per layer, each core projects its own nodes (x @ [Ws | Ws@bd(al) | Wd@bd(ar)]
    -> per-node table rows [fs | el] plus er), all-gathers the table, then
    processes its own edges grouped by 128-dst-node windows:
      - dma_gather of 512B table rows by src (two gathers: table split in two
        halves so indices fit int16),
      - onehot[j, m] = (dst_local[j] == m) via one is_equal per batch,
      - onehotT[m, j] likewise from a broadcast-replicated dst_local row;
        er_edge[j, h] = onehotT.T @ er_window via TensorE,
      - ee = exp(leaky_relu(el + er_edge)); msg = [ee * fs | ee],
      - seg[m, :] += onehot.T @ msg accumulated in PSUM over the group's
        chunks; last columns give the softmax denominator.
  * Layer-1 epilogue: h1 = elu(seg/denom + b1) kept in SBUF.
  * Layer-2 epilogue: mean over heads + log_softmax (Ln deferred to one pass).

Softmax max-subtraction is skipped: e = lrelu(el+er) with the given scales is
|e| < ~10, well within fp32 exp range, and alpha is shift-invariant.

Host work is index/schedule preparation only (sort, counts, layouts) plus
constant reparameterization (W @ blockdiag(a), np.tile of biases).
"""

import math
import os
import sys
from contextlib import ExitStack

import numpy as np

for _p in ("/opt/trn_rl_repo", "/root/.axon_site/_ro/trn_rl_repo"):
    if os.path.isdir(_p) and _p not in sys.path:
        sys.path.append(_p)

import ml_dtypes

import concourse.bass as bass
import concourse.tile as tile
from concourse import bacc, mybir
from concourse.bass_utils import run_bass_kernel_spmd

BF16 = ml_dtypes.bfloat16

N = 50000
E = 800000
F_IN = 128
H, D, C = 4, 32, 47
HD = H * D            # 128
HC = H * C            # 188
NEG_SLOPE = 0.2

NCORES = 8
P = 128
NPC = N // NCORES         # 6250
G = math.ceil(NPC / P)    # 49
NPAD = G * P              # 6272
NTAB = NPAD * NCORES      # 50176
S_SPLIT = NPAD * (NCORES // 2)   # 25088 (< int16 max on both halves)

W1 = HD + H               # 132
W2 = HC + H               # 192
ELEM = 256                # table row width (bf16) -> 512B, dma_gather aligned
OOR = 200.0               # out-of-window dst_local marker
BATCH = 8

LAST_EXEC_NS = None


def _schedule(src, dst):
    """Build per-core gather/one-hot schedule. Returns dict of arrays + KA/KB."""
    order = np.argsort(dst, kind="stable")
    s_src = src[order].astype(np.int64)
    s_dst = dst[order].astype(np.int64)

    core_of = s_dst // NPC
    g_of = (s_dst % NPC) // P
    pg_src = s_src + (s_src // NPC) * (NPAD - NPC)   # padded-global src row
    half = (pg_src >= S_SPLIT).astype(np.int64)
    win = (s_dst % NPC) % P

    cgh = (core_of * G + g_of) * 2 + half
    order2 = np.argsort(cgh, kind="stable")
    cgh = cgh[order2]
    pg_src = pg_src[order2]
    win = win[order2]

    counts = np.bincount(cgh, minlength=NCORES * G * 2).reshape(NCORES, G, 2)
    KA = int(math.ceil(counts[:, :, 0].max() / P))
    KB = int(math.ceil(counts[:, :, 1].max() / P))
    K = KA + KB

    starts = np.zeros(NCORES * G * 2 + 1, dtype=np.int64)
    np.cumsum(counts.ravel(), out=starts[1:])
    pos_in_run = np.arange(len(cgh)) - starts[cgh]

    # flat slot position within the [K*128] group stream
    base = np.where(cgh % 2 == 0, 0, KA * P)
    flat = base + pos_in_run
    cg = cgh // 2

    idx_flat = np.zeros((NCORES * G, K * P), dtype=np.int64)   # dummy row 0
    dstl_flat = np.full((NCORES * G, K * P), OOR, dtype=np.float32)
    idx_val = np.where(cgh % 2 == 0, pg_src, pg_src - S_SPLIT)
    idx_flat[cg, flat] = idx_val
    dstl_flat[cg, flat] = win

    idx_flat = idx_flat.reshape(NCORES, G, K, P)
    dstl_flat = dstl_flat.reshape(NCORES, G, K, P)

    def wrap(a):
        # [.., n] flat slot-major -> [.., 128, n/16] wrapped+replicated
        n = a.shape[-1]
        w = a.reshape(*a.shape[:-1], n // 16, 16)
        w = np.swapaxes(w, -1, -2)                    # [16, n/16]
        return np.tile(w, (1, 1, 8, 1)).astype(np.int16)  # [128, n/16]

    idxA_w = wrap(idx_flat[:, :, :KA, :].reshape(NCORES, G, KA * P))
    idxB_w = wrap(idx_flat[:, :, KA:, :].reshape(NCORES, G, KB * P))

    dstl_row = dstl_flat.reshape(NCORES, G, K * P).astype(BF16)
    dstl_col = np.swapaxes(dstl_flat, 2, 3).astype(BF16)      # [NC, G, 128, K]
    return dict(idxA_w=idxA_w, idxB_w=idxB_w, dstl_row=dstl_row,
                dstl_col=dstl_col, KA=KA, KB=KB)


def _blockdiag(a, hd, h, dim):
    out = np.zeros((hd, h), dtype=np.float32)
    for i in range(h):
        out[i * dim:(i + 1) * dim, i] = a[i]
    return out


_PHASES = ["proj1", "ag1", "edges1", "proj2", "edges2", "final", "full"]


def _build_program(KA, KB):
    K = KA + KB
    phase = os.environ.get("GAT_PHASE", "full")

    def go(p):
        # build phase p if the requested phase is at or beyond p
        return _PHASES.index(phase) >= _PHASES.index(p)
    nc = bacc.Bacc("TRN2", target_bir_lowering=False, debug=False,
                   num_devices=NCORES)
    dt = mybir.dt
    f32, bf16, i16 = dt.float32, dt.bfloat16, dt.int16

    def inp(name, shape, d=f32):
        return nc.dram_tensor(name, shape, d, kind="ExternalInput").ap()

    x_own = inp("x_own", [NPAD, F_IN])
    w1cat = inp("w1cat", [F_IN, W1 + H], bf16)
    w2cat = inp("w2cat", [F_IN, W2 + H], bf16)
    b1_t = inp("b1_t", [P, HD])
    b2m_t = inp("b2m_t", [P, C])
    iota_r = inp("iota_r", [P, P], bf16)       # iota_r[p, m] = m
    iota_c = inp("iota_c", [P, 1])             # iota_c[p, 0] = p (f32)
    ident_t = inp("ident_t", [P, P])
    idxA_in = inp("idxA_in", [G, P, KA * 8], i16)
    idxB_in = inp("idxB_in", [G, P, KB * 8], i16)
    dstl_row_in = inp("dstl_row_in", [G, K * P], bf16)
    dstl_col_in = inp("dstl_col_in", [G, P, K], bf16)

    y_out = nc.dram_tensor("y_out", [NPAD, C], f32, kind="ExternalOutput").ap()
    dbg = nc.dram_tensor("dbg", [NTAB, ELEM], bf16, kind="ExternalOutput").ap()

    tab1_own = nc.dram_tensor("tab1_own", [NPAD, ELEM], bf16).ap()
    tab1 = nc.dram_tensor("tab1", [NTAB, ELEM], bf16, addr_space="Shared").ap()
    tab1B = nc.dram_tensor("tab1B", [NTAB - S_SPLIT, ELEM], bf16).ap()
    er1_d = nc.dram_tensor("er1_d", [NPAD, H], f32).ap()
    tab2_own = nc.dram_tensor("tab2_own", [NPAD, ELEM], bf16).ap()
    tab2 = nc.dram_tensor("tab2", [NTAB, ELEM], bf16, addr_space="Shared").ap()
    tab2B = nc.dram_tensor("tab2B", [NTAB - S_SPLIT, ELEM], bf16).ap()
    er2_d = nc.dram_tensor("er2_d", [NPAD, H], f32).ap()

    with tile.TileContext(nc) as tc, ExitStack() as ctx:
        const = ctx.enter_context(tc.tile_pool(name="const", bufs=1))
        sb = ctx.enter_context(tc.tile_pool(name="sb", bufs=3))
        gat = ctx.enter_context(tc.tile_pool(name="gat", bufs=2))
        ps = ctx.enter_context(tc.tile_pool(name="ps", bufs=2, space="PSUM"))
        psg = ctx.enter_context(tc.tile_pool(name="psg", bufs=2, space="PSUM"))
        big = ctx.enter_context(tc.tile_pool(name="big", bufs=1))

        noconst = os.environ.get("GAT_NOCONST")
        iota = const.tile([P, P], bf16)
        nc.sync.dma_start(iota[:], iota_r[:])
        iotac = const.tile([P, 1], f32)
        nc.sync.dma_start(iotac[:], iota_c[:])
        ident = b1s = b2ms = w1 = w2 = None
        if not noconst:
            ident = const.tile([P, P], f32)
            nc.sync.dma_start(ident[:], ident_t[:])
            b1s = const.tile([P, HD], f32)
            nc.sync.dma_start(b1s[:], b1_t[:])
            b2ms = const.tile([P, C], f32)
            nc.sync.dma_start(b2ms[:], b2m_t[:])
            w1 = const.tile([P, W1 + H], bf16)
            nc.sync.dma_start(w1[:], w1cat[:])
            w2 = const.tile([P, W2 + H], bf16)
            nc.sync.dma_start(w2[:], w2cat[:])

        h1 = zs = ss = None
        if not os.environ.get("GAT_NOBIG"):
            h1 = big.tile([P, G, F_IN], f32)
            zs = big.tile([P, G, C], f32)
            ss = big.tile([P, G], f32)

        # ---------------- projection ----------------
        def project(src_tile_of, wcat, width, tab_own_d, er_d):
            for g in range(G):
                xt = src_tile_of(g)
                xT_ps = ps.tile([F_IN, P], f32, space="PSUM", tag="xT_ps")
                nc.tensor.transpose(xT_ps[:], xt[:], ident[:])
                xT = sb.tile([F_IN, P], bf16, tag="xT")
                nc.vector.tensor_copy(xT[:], xT_ps[:])
                pr = ps.tile([P, width + H], f32, space="PSUM", tag="proj")
                nc.tensor.matmul(pr[:], lhsT=xT[:], rhs=wcat[:, :width + H],
                                 start=True, stop=True)
                tb = sb.tile([P, width], bf16, tag="tabrow")
                nc.vector.tensor_copy(tb[:], pr[:, :width])
                nc.sync.dma_start(tab_own_d[g * P:(g + 1) * P, :width], tb[:])
                er = sb.tile([P, H], f32, tag="errow")
                nc.vector.tensor_copy(er[:], pr[:, width:width + H])
                nc.sync.dma_start(er_d[g * P:(g + 1) * P, :], er[:])

        def x_tile(g):
            t = sb.tile([P, F_IN], f32, tag="xload")
            nc.sync.dma_start(t[:], x_own[g * P:(g + 1) * P, :])
            return t

        if os.environ.get("GAT_NOPROJ"):
            for g in range(G):
                nc.gpsimd.dma_start(out=tab1_own[g * P:(g + 1) * P, :F_IN],
                                    in_=x_own[g * P:(g + 1) * P, :])
                nc.sync.dma_start(out=er1_d[g * P:(g + 1) * P, :],
                                  in_=x_own[g * P:(g + 1) * P, :H])
        else:
            project(x_tile, w1, W1, tab1_own, er1_d)

        if phase == "proj1":
            nc.sync.dma_start(dbg[:NPAD, :], tab1_own[:])
        if go("ag1"):
            nc.gpsimd.collective_compute(
            "AllGather", mybir.AluOpType.bypass,
                replica_groups=[list(range(NCORES))],
                ins=[tab1_own[:]], outs=[tab1[:]])
            nc.sync.dma_start(tab1B[:], tab1[S_SPLIT:, :])
        if phase == "ag1":
            nc.sync.dma_start(dbg[:], tab1[:])

        # ---------------- edge phase ----------------
        edgelvl = int(os.environ.get("GAT_EDGELVL", "9"))

        def edge_phase(tab_full, tab_B, er_d, width, out_cb):
            nb = math.ceil(K / BATCH)
            for g in range(G):
                idxA_t = sb.tile([P, KA * 8], i16, tag="idxA")
                nc.sync.dma_start(idxA_t[:], idxA_in[g])
                idxB_t = sb.tile([P, KB * 8], i16, tag="idxB")
                nc.sync.dma_start(idxB_t[:], idxB_in[g])
                dcol = sb.tile([P, K], bf16, tag="dcol")
                nc.sync.dma_start(dcol[:], dstl_col_in[g])
                drep = sb.tile([P, K * P], bf16, tag="drep")
                if os.environ.get("GAT_NOBCAST"):
                    nc.sync.dma_start(drep[:1, :], dstl_row_in[g:g + 1, :])
                else:
                    nc.sync.dma_start(
                        drep[:], dstl_row_in[g:g + 1, :].to_broadcast([P, K * P]))
                erw_f = sb.tile([P, H], f32, tag="erwf")
                nc.sync.dma_start(erw_f[:], er_d[g * P:(g + 1) * P, :])
                erw = sb.tile([P, H], bf16, tag="erw")
                nc.vector.tensor_copy(erw[:], erw_f[:])

                gt = gat.tile([P, K, ELEM], bf16, tag="gt")
                if edgelvl >= 1:
                    nc.gpsimd.dma_gather(
                        out_ap=gt[:, :KA, :], in_ap=tab_full[:S_SPLIT, :],
                        idxs_ap=idxA_t[:], num_idxs=KA * P,
                        num_idxs_reg=KA * P, elem_size=ELEM,
                        single_packet=False)
                    nc.gpsimd.dma_gather(
                        out_ap=gt[:, KA:, :], in_ap=tab_B[:],
                        idxs_ap=idxB_t[:], num_idxs=KB * P,
                        num_idxs_reg=KB * P, elem_size=ELEM,
                        single_packet=False)

                seg = None
                if not os.environ.get("GAT_NOSEG"):
                    seg = psg.tile([P, width], f32, space="PSUM", tag="seg")
                for b in range(nb):
                    k0 = b * BATCH
                    kb = min(BATCH, K - k0)
                    if edgelvl < 2:
                        continue
                    oh = sb.tile([P, BATCH, P], bf16, tag="oh")
                    nc.vector.tensor_tensor(
                        out=oh[:, :kb, :],
                        in0=dcol[:, k0:k0 + kb, None].to_broadcast([P, kb, P]),
                        in1=iota[:, None, :].to_broadcast([P, kb, P]),
                        op=mybir.AluOpType.is_equal)
                    if edgelvl < 3:
                        continue
                    ohT = sb.tile([P, BATCH, P], bf16, tag="ohT")
                    nc.vector.tensor_scalar(
                        out=ohT[:, :kb, :],
                        in0=drep[:, k0 * P:(k0 + kb) * P].rearrange(
                            "p (k j) -> p k j", k=kb),
                        scalar1=iotac[:, :1], scalar2=None,
                        op0=mybir.AluOpType.is_equal)
                    if edgelvl < 4:
                        continue
                    erp = psg.tile([P, BATCH, H], f32, space="PSUM", tag="erp")
                    for c in range(kb):
                        nc.tensor.matmul(
                            erp[:, c, :], lhsT=ohT[:, c, :], rhs=erw[:],
                            start=True, stop=True)
                    if edgelvl < 5:
                        continue
                    ev = sb.tile([P, BATCH, H], f32, tag="ev")
                    nc.vector.tensor_tensor(
                        out=ev[:, :kb, :],
                        in0=gt[:, k0:k0 + kb, width - H:width],
                        in1=erp[:, :kb, :], op=mybir.AluOpType.add)
                    nc.vector.scalar_tensor_tensor(
                        out=ev[:, :kb, :], in0=ev[:, :kb, :], scalar=NEG_SLOPE,
                        in1=ev[:, :kb, :],
                        op0=mybir.AluOpType.mult, op1=mybir.AluOpType.max)
                    if edgelvl < 6:
                        continue
                    mt = sb.tile([P, BATCH, width], bf16, tag="mt")
                    nc.scalar.activation(
                        mt[:, :kb, width - H:width], ev[:, :kb, :],
                        mybir.ActivationFunctionType.Exp)
                    nc.vector.tensor_tensor(
                        out=mt[:, :kb, :width - H].rearrange(
                            "p k (h d) -> p k h d", h=H),
                        in0=gt[:, k0:k0 + kb, :width - H].rearrange(
                            "p k (h d) -> p k h d", h=H),
                        in1=mt[:, :kb, width - H:width, None].to_broadcast(
                            [P, kb, H, (width - H) // H]),
                        op=mybir.AluOpType.mult)
                    if edgelvl < 7:
                        continue
                    for c in range(kb):
                        nc.tensor.matmul(
                            seg[:], lhsT=oh[:, c, :], rhs=mt[:, c, :],
                            start=(b == 0 and c == 0),
                            stop=(b == nb - 1 and c == kb - 1))
                if edgelvl >= 7:
                    out_cb(g, seg)

        def l1_out(g, seg):
            dn = sb.tile([P, H], f32, tag="dn")
            nc.vector.tensor_scalar_max(dn[:], seg[:, HD:HD + H], 1e-30)
            rd = sb.tile([P, H], f32, tag="rd")
            nc.vector.reciprocal(rd[:], dn[:])
            ht = sb.tile([P, F_IN], f32, tag="ht")
            nc.vector.tensor_tensor(
                out=ht[:].rearrange("p (h d) -> p h d", h=H),
                in0=seg[:, :HD].rearrange("p (h d) -> p h d", h=H),
                in1=rd[:, :, None].to_broadcast([P, H, D]),
                op=mybir.AluOpType.mult)
            nc.vector.tensor_tensor(
                out=ht[:], in0=ht[:], in1=b1s[:], op=mybir.AluOpType.add)
            mn = sb.tile([P, F_IN], f32, tag="mn")
            nc.vector.tensor_scalar_min(mn[:], ht[:], 0.0)
            nc.scalar.activation(mn[:], mn[:], mybir.ActivationFunctionType.Exp)
            nc.vector.scalar_tensor_tensor(
                out=h1[:, g, :], in0=mn[:], scalar=-1.0, in1=ht[:],
                op0=mybir.AluOpType.add, op1=mybir.AluOpType.max)

        if go("edges1"):
            edge_phase(tab1, tab1B, er1_d, W1, l1_out)
        if phase == "edges1" and edgelvl >= 7:
            for g in range(G):
                hd_t = sb.tile([P, F_IN], bf16, tag="hdump")
                nc.vector.tensor_copy(hd_t[:], h1[:, g, :])
                nc.sync.dma_start(dbg[g * P:(g + 1) * P, :F_IN], hd_t[:])

        def h1_tile(g):
            return h1[:, g, :]

        if go("proj2"):
            project(h1_tile, w2, W2, tab2_own, er2_d)
            nc.gpsimd.collective_compute(
                "AllGather", mybir.AluOpType.bypass,
                replica_groups=[list(range(NCORES))],
                ins=[tab2_own[:]], outs=[tab2[:]])
            nc.sync.dma_start(tab2B[:], tab2[S_SPLIT:, :])

        def l2_out(g, seg):
            dn = sb.tile([P, H], f32, tag="dn2")
            nc.vector.tensor_scalar_max(dn[:], seg[:, HC:HC + H], 1e-30)
            rd = sb.tile([P, H], f32, tag="rd2")
            nc.vector.reciprocal(rd[:], dn[:])
            nc.vector.tensor_scalar_mul(rd[:], rd[:], 1.0 / H)
            z = sb.tile([P, HC], f32, tag="z")
            nc.vector.tensor_tensor(
                out=z[:].rearrange("p (h c) -> p h c", h=H),
                in0=seg[:, :HC].rearrange("p (h c) -> p h c", h=H),
                in1=rd[:, :, None].to_broadcast([P, H, C]),
                op=mybir.AluOpType.mult)
            z4 = sb.tile([P, C], f32, tag="z4")
            nc.vector.reduce_sum(
                z4[:], z[:].rearrange("p (h c) -> p c h", h=H),
                axis=mybir.AxisListType.X)
            nc.vector.tensor_tensor(
                out=z4[:], in0=z4[:], in1=b2ms[:], op=mybir.AluOpType.add)
            zm = sb.tile([P, 1], f32, tag="zm")
            nc.vector.reduce_max(zm[:], z4[:], axis=mybir.AxisListType.X)
            nc.vector.tensor_scalar(
                out=zs[:, g, :], in0=z4[:], scalar1=zm[:, :1], scalar2=None,
                op0=mybir.AluOpType.subtract)
            es = sb.tile([P, C], f32, tag="es")
            nc.scalar.activation(es[:], zs[:, g, :],
                                 mybir.ActivationFunctionType.Exp,
                                 accum_out=ss[:, g:g + 1])

        if go("edges2"):
            edge_phase(tab2, tab2B, er2_d, W2, l2_out)

        if go("final"):
            lg = sb.tile([P, G], f32, tag="lg")
            nc.scalar.activation(lg[:], ss[:], mybir.ActivationFunctionType.Ln)
            for g in range(G):
                yt = sb.tile([P, C], f32, tag="yt")
                nc.vector.tensor_scalar(
                    out=yt[:], in0=zs[:, g, :], scalar1=lg[:, g:g + 1],
                    scalar2=None, op0=mybir.AluOpType.subtract)
                nc.sync.dma_start(y_out[g * P:(g + 1) * P, :], yt[:])

    nc.compile()
    return nc


def kernel(x, src, dst, W1s, W1d, al1, ar1, b1, W2s, W2d, al2, ar2, b2):
    global LAST_EXEC_NS
    x = np.asarray(x, dtype=np.float32)
    src = np.asarray(src, dtype=np.int32)
    dst = np.asarray(dst, dtype=np.int32)

    sch = _schedule(src, dst)
    KA, KB = sch["KA"], sch["KB"]

    def wcat(Ws, Wd, al, ar, hd, h, dim):
        Ws = np.asarray(Ws, np.float32)
        Wd = np.asarray(Wd, np.float32)
        wel = Ws @ _blockdiag(np.asarray(al, np.float32), hd, h, dim)
        wer = Wd @ _blockdiag(np.asarray(ar, np.float32), hd, h, dim)
        return np.concatenate([Ws, wel, wer], axis=1)

    w1c = wcat(W1s, W1d, al1, ar1, HD, H, D).astype(BF16)
    w2c = wcat(W2s, W2d, al2, ar2, HC, H, C).astype(BF16)

    iota_r = np.tile(np.arange(P, dtype=np.float32), (P, 1)).astype(BF16)
    iota_c = np.arange(P, dtype=np.float32)[:, None]
    ident_np = np.eye(P, dtype=np.float32)
    b1_np = np.tile(np.asarray(b1, np.float32)[None, :], (P, 1))
    b2m_np = np.tile(np.asarray(b2, np.float32).reshape(H, C).mean(0)[None, :],
                     (P, 1))

    x_pad = np.zeros((NCORES, NPAD, F_IN), np.float32)
    x_pad[:, :NPC, :] = x.reshape(NCORES, NPC, F_IN)

    nc = _build_program(KA, KB)

    in_maps = []
    for c in range(NCORES):
        in_maps.append({
            "x_own": x_pad[c],
            "w1cat": w1c, "w2cat": w2c,
            "b1_t": b1_np, "b2m_t": b2m_np,
            "iota_r": iota_r, "iota_c": iota_c, "ident_t": ident_np,
            "idxA_in": sch["idxA_w"][c], "idxB_in": sch["idxB_w"][c],
            "dstl_row_in": sch["dstl_row"][c],
            "dstl_col_in": sch["dstl_col"][c],
        })

    res = run_bass_kernel_spmd(nc, in_maps, list(range(NCORES)),
                               trace=bool(os.environ.get("GAT_TRACE")))
    LAST_EXEC_NS = res.exec_time_ns
    out = np.concatenate(
        [res.results[c]["y_out"][:NPC] for c in range(NCORES)], axis=0)
    return out.astype(np.float32)



# revision 4
# speedup vs baseline: 1.0553x; 1.0553x over previous
"""Two-layer GAT (DGL GATConv) on 8 Trainium2 NeuronCores — v2.

Strategy (edge-parallel, dst-sorted), changes vs v1:
  * One-hot window matrices (oh: [slot j, dst m], ohT: [m, j]) are built on
    the HOST and DMA'd as bf16, eliminating all DVE is_equal work (the v1
    Vector bottleneck: slow IS_EQ uops + per-partition PTR-scalar reads).
  * The per-layer table AllGather is split into two half-collectives
    (rows [0:3200) and [3200:6272) of each core's slab) so A-half gathers
    start while the B half is still in flight.  The halves also give the
    int16 index split for dma_gather (25600 / 24576 rows < 2^15).
  * dma_gather is issued per group-PAIR per half (fewer SWDGE fixed costs).
  * Small epilogue ops moved off DVE onto ScalarE activations with
    per-partition bias APs (log-softmax shift, PSUM evacuations).

Softmax max-subtraction is skipped: e = lrelu(el+er) with the given scales
is small, and alpha is shift-invariant.
"""

import math
import os
import sys
from contextlib import ExitStack

import numpy as np

for _p in ("/opt/trn_rl_repo", "/root/.axon_site/_ro/trn_rl_repo"):
    if os.path.isdir(_p) and _p not in sys.path:
        sys.path.append(_p)

import ml_dtypes

import concourse.bass as bass
import concourse.tile as tile
from concourse import bacc, mybir
from concourse.bass_utils import run_bass_kernel_spmd

BF16 = ml_dtypes.bfloat16

N = 50000
E = 800000
F_IN = 128
H, D, C = 4, 32, 47
HD = H * D            # 128
HC = H * C            # 188
NEG_SLOPE = 0.2

NCORES = 8
P = 128
NPC = N // NCORES         # 6250
G = math.ceil(NPC / P)    # 49
NPAD = G * P              # 6272
GA = 25                   # groups in half A
HSPLIT = GA * P           # 3200 rows per core in half A
HB = NPAD - HSPLIT        # 3072 rows per core in half B
NTA = HSPLIT * NCORES     # 25600 (< 2^15)
NTB = HB * NCORES         # 24576 (< 2^15)

W1 = HD + H               # 132
W2 = HC + H               # 192
ELEM = 256                # gathered row width in bf16 -> 512B

LAST_EXEC_NS = None


def _schedule(src, dst):
    """Edge schedule: per (core, group, half) slot runs + one-hot tensors."""
    order = np.argsort(dst, kind="stable")
    s_src = src[order].astype(np.int64)
    s_dst = dst[order].astype(np.int64)

    core_of = s_dst // NPC
    g_of = (s_dst % NPC) // P
    win = (s_dst % NPC) % P

    src_c = s_src // NPC               # owner core of src
    src_i = s_src % NPC                # within-core index (< 6250)
    half = (src_i >= HSPLIT).astype(np.int64)
    idx_val = np.where(half == 0, src_c * HSPLIT + src_i,
                       src_c * HB + (src_i - HSPLIT))

    cgh = (core_of * G + g_of) * 2 + half
    order2 = np.argsort(cgh, kind="stable")
    cgh = cgh[order2]
    idx_val = idx_val[order2]
    win = win[order2]

    counts = np.bincount(cgh, minlength=NCORES * G * 2).reshape(NCORES, G, 2)
    KA = int(math.ceil(counts[:, :, 0].max() / P))
    KB = int(math.ceil(counts[:, :, 1].max() / P))
    K = KA + KB

    starts = np.zeros(NCORES * G * 2 + 1, dtype=np.int64)
    np.cumsum(counts.ravel(), out=starts[1:])
    pos_in_run = np.arange(len(cgh)) - starts[cgh]

    base = np.where(cgh % 2 == 0, 0, KA * P)
    flat = base + pos_in_run
    cg = cgh // 2

    idx_flat = np.zeros((NCORES * G, K * P), dtype=np.int64)   # pad -> row 0
    dstl_flat = np.full((NCORES * G, K * P), -1, dtype=np.int64)  # pad -> -1
    idx_flat[cg, flat] = idx_val
    dstl_flat[cg, flat] = win

    idx_flat = idx_flat.reshape(NCORES, G, K, P)
    dstl_flat = dstl_flat.reshape(NCORES, G, K, P)

    def wrap(a):
        # [NC, G, n] slot-major -> [NC, G, 128, n/16] wrapped+replicated
        n = a.shape[-1]
        w = a.reshape(*a.shape[:-1], n // 16, 16)
        w = np.swapaxes(w, -1, -2)                    # [.., 16, n/16]
        return np.tile(w, (1, 1, 8, 1)).astype(np.int16)

    idxA_w = wrap(idx_flat[:, :, :KA, :].reshape(NCORES, G, KA * P))
    idxB_w = wrap(idx_flat[:, :, KA:, :].reshape(NCORES, G, KB * P))

    # one-hots, built per core to bound peak memory
    m_ar = np.arange(P, dtype=np.int64)
    oh = np.empty((NCORES, G, P, K * P), dtype=BF16)
    ohT = np.empty((NCORES, G, P, K * P), dtype=BF16)
    for c in range(NCORES):
        d = dstl_flat[c]                                   # [G, K, P]
        eq = (d[:, :, :, None] == m_ar).astype(BF16)       # [G, K, Pj, Pm]
        # oh tile: [j, (chunk, m)]
        oh[c] = eq.transpose(0, 2, 1, 3).reshape(G, P, K * P)
        # ohT tile: [m, (chunk, j)]
        ohT[c] = eq.transpose(0, 3, 1, 2).reshape(G, P, K * P)

    return dict(idxA_w=idxA_w, idxB_w=idxB_w, oh=oh, ohT=ohT, KA=KA, KB=KB)


def _blockdiag(a, hd, h, dim):
    out = np.zeros((hd, h), dtype=np.float32)
    for i in range(h):
        out[i * dim:(i + 1) * dim, i] = a[i]
    return out


def _build_program(KA, KB):
    K = KA + KB
    nc = bacc.Bacc("TRN2", target_bir_lowering=False, debug=False,
                   num_devices=NCORES)
    dt = mybir.dt
    f32, bf16, i16 = dt.float32, dt.bfloat16, dt.int16

    def inp(name, shape, d=f32):
        return nc.dram_tensor(name, shape, d, kind="ExternalInput").ap()

    x_own = inp("x_own", [NPAD, F_IN])
    w1cat = inp("w1cat", [F_IN, W1 + H], bf16)
    w2cat = inp("w2cat", [F_IN, W2 + H], bf16)
    b1_t = inp("b1_t", [P, HD])
    b2m_t = inp("b2m_t", [P, C])
    ident_t = inp("ident_t", [P, P])
    idxA_in = inp("idxA_in", [G, P, KA * 8], i16)
    idxB_in = inp("idxB_in", [G, P, KB * 8], i16)
    oh_in = inp("oh_in", [G, P, K * P], bf16)
    ohT_in = inp("ohT_in", [G, P, K * P], bf16)

    y_out = nc.dram_tensor("y_out", [NPAD, C], f32, kind="ExternalOutput").ap()

    tab1_own = nc.dram_tensor("tab1_own", [NPAD, ELEM], bf16).ap()
    tab1A = nc.dram_tensor("tab1A", [NTA, ELEM], bf16, addr_space="Shared").ap()
    tab1B = nc.dram_tensor("tab1B", [NTB, ELEM], bf16, addr_space="Shared").ap()
    er1_d = nc.dram_tensor("er1_d", [NPAD, H], f32).ap()
    tab2_own = nc.dram_tensor("tab2_own", [NPAD, ELEM], bf16).ap()
    tab2A = nc.dram_tensor("tab2A", [NTA, ELEM], bf16, addr_space="Shared").ap()
    tab2B = nc.dram_tensor("tab2B", [NTB, ELEM], bf16, addr_space="Shared").ap()
    er2_d = nc.dram_tensor("er2_d", [NPAD, H], f32).ap()

    # group pairs for gather batching
    pairs = [(g, g + 1) if g + 1 < G else (g,) for g in range(0, G, 2)]

    with tile.TileContext(nc) as tc, ExitStack() as ctx:
        const = ctx.enter_context(tc.tile_pool(name="const", bufs=1))
        sb = ctx.enter_context(tc.tile_pool(name="sb", bufs=3))
        ohp = ctx.enter_context(tc.tile_pool(name="ohp", bufs=2))
        gat = ctx.enter_context(tc.tile_pool(name="gat", bufs=2))
        ps = ctx.enter_context(tc.tile_pool(name="ps", bufs=2, space="PSUM"))
        psg = ctx.enter_context(tc.tile_pool(name="psg", bufs=2, space="PSUM"))
        big = ctx.enter_context(tc.tile_pool(name="big", bufs=1))

        ident = const.tile([P, P], f32)
        nc.sync.dma_start(ident[:], ident_t[:])
        b1s = const.tile([P, HD], f32)
        nc.sync.dma_start(b1s[:], b1_t[:])
        b2ms = const.tile([P, C], f32)
        nc.sync.dma_start(b2ms[:], b2m_t[:])
        w1 = const.tile([P, W1 + H], bf16)
        nc.sync.dma_start(w1[:], w1cat[:])
        w2 = const.tile([P, W2 + H], bf16)
        nc.sync.dma_start(w2[:], w2cat[:])

        h1 = big.tile([P, G, F_IN], f32)
        zs = big.tile([P, G, C], f32)
        ss = big.tile([P, G], f32)

        # ---------------- projection (split for collective overlap) --------
        def project(src_tile_of, wcat, width, tab_own_d, er_d, glo, ghi):
            for g in range(glo, ghi):
                xt = src_tile_of(g)
                xT_ps = ps.tile([F_IN, P], f32, space="PSUM", tag="xT_ps")
                nc.tensor.transpose(xT_ps[:], xt[:], ident[:])
                xT = sb.tile([F_IN, P], bf16, tag="xT")
                nc.vector.tensor_copy(xT[:], xT_ps[:])
                pr = ps.tile([P, width + H], f32, space="PSUM", tag="proj")
                nc.tensor.matmul(pr[:], lhsT=xT[:], rhs=wcat[:, :width + H],
                                 start=True, stop=True)
                tb = sb.tile([P, width], bf16, tag="tabrow")
                nc.scalar.activation(tb[:], pr[:, :width],
                                     mybir.ActivationFunctionType.Copy)
                nc.sync.dma_start(tab_own_d[g * P:(g + 1) * P, :width], tb[:])
                er = sb.tile([P, H], f32, tag="errow")
                nc.scalar.activation(er[:], pr[:, width:width + H],
                                     mybir.ActivationFunctionType.Copy)
                nc.sync.dma_start(er_d[g * P:(g + 1) * P, :], er[:])

        def x_tile(g):
            t = sb.tile([P, F_IN], f32, tag="xload")
            nc.sync.dma_start(t[:], x_own[g * P:(g + 1) * P, :])
            return t

        def proj_and_gather_layer(src_tile_of, wcat, width, tab_own_d,
                                  tabA, tabB, er_d):
            project(src_tile_of, wcat, width, tab_own_d, er_d, 0, GA)
            nc.gpsimd.collective_compute(
                "AllGather", mybir.AluOpType.bypass,
                replica_groups=[list(range(NCORES))],
                ins=[tab_own_d[:HSPLIT, :]], outs=[tabA[:]])
            project(src_tile_of, wcat, width, tab_own_d, er_d, GA, G)
            nc.gpsimd.collective_compute(
                "AllGather", mybir.AluOpType.bypass,
                replica_groups=[list(range(NCORES))],
                ins=[tab_own_d[HSPLIT:, :]], outs=[tabB[:]])

        # ---------------- edge phase ----------------
        def edge_phase(tabA, tabB, er_d, width, out_cb):
            for pair in pairs:
                npair = len(pair)
                # gathers: one per half covering the whole pair
                gts = {}
                for hkey, (tab, Kh, idx_in) in (
                        ("A", (tabA, KA, idxA_in)),
                        ("B", (tabB, KB, idxB_in))):
                    it = sb.tile([P, npair * Kh * 8], i16, tag=f"idx{hkey}")
                    for gi, g in enumerate(pair):
                        nc.sync.dma_start(
                            it[:, gi * Kh * 8:(gi + 1) * Kh * 8], idx_in[g])
                    gt = gat.tile([P, npair * Kh, ELEM], bf16, tag=f"gt{hkey}")
                    nc.gpsimd.dma_gather(
                        out_ap=gt[:], in_ap=tab[:],
                        idxs_ap=it[:], num_idxs=npair * Kh * P,
                        num_idxs_reg=npair * Kh * P, elem_size=ELEM,
                        single_packet=False)
                    gts[hkey] = gt

                for gi, g in enumerate(pair):
                    oh_t = ohp.tile([P, K, P], bf16, tag="oh")
                    nc.sync.dma_start(
                        oh_t[:].rearrange("p k m -> p (k m)"), oh_in[g])
                    ohT_t = ohp.tile([P, K, P], bf16, tag="ohT")
                    nc.sync.dma_start(
                        ohT_t[:].rearrange("p k m -> p (k m)"), ohT_in[g])
                    erw_f = sb.tile([P, H], f32, tag="erwf")
                    nc.sync.dma_start(erw_f[:], er_d[g * P:(g + 1) * P, :])
                    erw = sb.tile([P, H], bf16, tag="erw")
                    nc.scalar.activation(erw[:], erw_f[:],
                                         mybir.ActivationFunctionType.Copy)

                    seg = psg.tile([P, width], f32, space="PSUM", tag="seg")
                    for bi, (hkey, Kh, coh) in enumerate(
                            (("A", KA, 0), ("B", KB, KA))):
                        gt = gts[hkey]
                        c0 = gi * Kh
                        erp = psg.tile([P, Kh, H], f32, space="PSUM",
                                       tag="erp")
                        for c in range(Kh):
                            nc.tensor.matmul(
                                erp[:, c, :], lhsT=ohT_t[:, coh + c, :],
                                rhs=erw[:], start=True, stop=True)
                        ev = sb.tile([P, Kh, H], f32, tag=f"ev{hkey}")
                        nc.vector.tensor_tensor(
                            out=ev[:],
                            in0=gt[:, c0:c0 + Kh, width - H:width],
                            in1=erp[:], op=mybir.AluOpType.add)
                        nc.vector.scalar_tensor_tensor(
                            out=ev[:], in0=ev[:], scalar=NEG_SLOPE,
                            in1=ev[:],
                            op0=mybir.AluOpType.mult, op1=mybir.AluOpType.max)
                        mt = sb.tile([P, Kh, width], bf16, tag=f"mt{hkey}")
                        nc.scalar.activation(
                            mt[:, :, width - H:width], ev[:],
                            mybir.ActivationFunctionType.Exp)
                        nc.vector.tensor_tensor(
                            out=mt[:, :, :width - H].rearrange(
                                "p k (h d) -> p k h d", h=H),
                            in0=gt[:, c0:c0 + Kh, :width - H].rearrange(
                                "p k (h d) -> p k h d", h=H),
                            in1=mt[:, :, width - H:width, None].to_broadcast(
                                [P, Kh, H, (width - H) // H]),
                            op=mybir.AluOpType.mult)
                        for c in range(Kh):
                            nc.tensor.matmul(
                                seg[:], lhsT=oh_t[:, coh + c, :],
                                rhs=mt[:, c, :],
                                start=(bi == 0 and c == 0),
                                stop=(bi == 1 and c == Kh - 1))
                    out_cb(g, seg)

        # ---------------- layer epilogues ----------------
        def l1_out(g, seg):
            dn = sb.tile([P, H], f32, tag="dn")
            nc.vector.tensor_scalar_max(dn[:], seg[:, HD:HD + H], 1e-30)
            rd = sb.tile([P, H], f32, tag="rd")
            nc.vector.reciprocal(rd[:], dn[:])
            ht = sb.tile([P, F_IN], f32, tag="ht")
            nc.vector.tensor_tensor(
                out=ht[:].rearrange("p (h d) -> p h d", h=H),
                in0=seg[:, :HD].rearrange("p (h d) -> p h d", h=H),
                in1=rd[:, :, None].to_broadcast([P, H, D]),
                op=mybir.AluOpType.mult)
            nc.vector.tensor_tensor(
                out=ht[:], in0=ht[:], in1=b1s[:], op=mybir.AluOpType.add)
            mn = sb.tile([P, F_IN], f32, tag="mn")
            nc.vector.tensor_scalar_min(mn[:], ht[:], 0.0)
            nc.scalar.activation(mn[:], mn[:], mybir.ActivationFunctionType.Exp)
            nc.vector.scalar_tensor_tensor(
                out=h1[:, g, :], in0=mn[:], scalar=-1.0, in1=ht[:],
                op0=mybir.AluOpType.add, op1=mybir.AluOpType.max)

        def h1_tile(g):
            return h1[:, g, :]

        def l2_out(g, seg):
            dn = sb.tile([P, H], f32, tag="dn2")
            nc.vector.tensor_scalar_max(dn[:], seg[:, HC:HC + H], 1e-30)
            rd = sb.tile([P, H], f32, tag="rd2")
            nc.vector.reciprocal(rd[:], dn[:])
            nc.vector.tensor_scalar_mul(rd[:], rd[:], 1.0 / H)
            z = sb.tile([P, HC], f32, tag="z")
            nc.vector.tensor_tensor(
                out=z[:].rearrange("p (h c) -> p h c", h=H),
                in0=seg[:, :HC].rearrange("p (h c) -> p h c", h=H),
                in1=rd[:, :, None].to_broadcast([P, H, C]),
                op=mybir.AluOpType.mult)
            z4 = sb.tile([P, C], f32, tag="z4")
            nc.vector.reduce_sum(
                z4[:], z[:].rearrange("p (h c) -> p c h", h=H),
                axis=mybir.AxisListType.X)
            nc.vector.tensor_tensor(
                out=z4[:], in0=z4[:], in1=b2ms[:], op=mybir.AluOpType.add)
            zm = sb.tile([P, 1], f32, tag="zm")
            nc.vector.reduce_max(zm[:], z4[:], axis=mybir.AxisListType.X)
            nzm = sb.tile([P, 1], f32, tag="nzm")
            nc.vector.tensor_scalar_mul(nzm[:], zm[:], -1.0)
            nc.scalar.activation(zs[:, g, :], z4[:],
                                 mybir.ActivationFunctionType.Identity,
                                 bias=nzm[:, :1])
            es = sb.tile([P, C], f32, tag="es")
            nc.scalar.activation(es[:], zs[:, g, :],
                                 mybir.ActivationFunctionType.Exp,
                                 accum_out=ss[:, g:g + 1])

        # ---------------- run the two layers ----------------
        proj_and_gather_layer(x_tile, w1, W1, tab1_own, tab1A, tab1B, er1_d)
        edge_phase(tab1A, tab1B, er1_d, W1, l1_out)

        proj_and_gather_layer(h1_tile, w2, W2, tab2_own, tab2A, tab2B, er2_d)
        edge_phase(tab2A, tab2B, er2_d, W2, l2_out)

        lg = sb.tile([P, G], f32, tag="lg")
        nc.scalar.activation(lg[:], ss[:], mybir.ActivationFunctionType.Ln)
        nlg = sb.tile([P, G], f32, tag="nlg")
        nc.vector.tensor_scalar_mul(nlg[:], lg[:], -1.0)
        for g in range(G):
            yt = sb.tile([P, C], f32, tag="yt")
            nc.scalar.activation(yt[:], zs[:, g, :],
                                 mybir.ActivationFunctionType.Identity,
                                 bias=nlg[:, g:g + 1])
            nc.sync.dma_start(y_out[g * P:(g + 1) * P, :], yt[:])

    nc.compile()
    return nc


def kernel(x, src, dst, W1s, W1d, al1, ar1, b1, W2s, W2d, al2, ar2, b2):
    global LAST_EXEC_NS
    x = np.asarray(x, dtype=np.float32)
    src = np.asarray(src, dtype=np.int32)
    dst = np.asarray(dst, dtype=np.int32)

    sch = _schedule(src, dst)
    KA, KB = sch["KA"], sch["KB"]

    def wcat(Ws, Wd, al, ar, hd, h, dim):
        Ws = np.asarray(Ws, np.float32)
        Wd = np.asarray(Wd, np.float32)
        wel = Ws @ _blockdiag(np.asarray(al, np.float32), hd, h, dim)
        wer = Wd @ _blockdiag(np.asarray(ar, np.float32), hd, h, dim)
        return np.concatenate([Ws, wel, wer], axis=1)

    w1c = wcat(W1s, W1d, al1, ar1, HD, H, D).astype(BF16)
    w2c = wcat(W2s, W2d, al2, ar2, HC, H, C).astype(BF16)

    ident_np = np.eye(P, dtype=np.float32)
    b1_np = np.tile(np.asarray(b1, np.float32)[None, :], (P, 1))
    b2m_np = np.tile(np.asarray(b2, np.float32).reshape(H, C).mean(0)[None, :],
                     (P, 1))

    x_pad = np.zeros((NCORES, NPAD, F_IN), np.float32)
    x_pad[:, :NPC, :] = x.reshape(NCORES, NPC, F_IN)

    nc = _build_program(KA, KB)

    in_maps = []
    for c in range(NCORES):
        in_maps.append({
            "x_own": x_pad[c],
            "w1cat": w1c, "w2cat": w2c,
            "b1_t": b1_np, "b2m_t": b2m_np,
            "ident_t": ident_np,
            "idxA_in": sch["idxA_w"][c], "idxB_in": sch["idxB_w"][c],
            "oh_in": sch["oh"][c], "ohT_in": sch["ohT"][c],
        })

    res = run_bass_kernel_spmd(nc, in_maps, list(range(NCORES)),
                               trace=bool(os.environ.get("GAT_TRACE")))
    LAST_EXEC_NS = res.exec_time_ns
    out = np.concatenate(
        [res.results[c]["y_out"][:NPC] for c in range(NCORES)], axis=0)
    return out.astype(np.float32)


# revision 6
# speedup vs baseline: 1.1342x; 1.0748x over previous
"""Two-layer GAT (DGL GATConv) on 8 Trainium2 NeuronCores — v3.

v3 over v2:
  * Epilogues batched: per-group seg PSUM is stashed (one ScalarE copy) into
    a [P, G, W] staging tile; softmax-normalize / elu / log-softmax run as a
    handful of LARGE DVE ops per block of groups instead of ~6 tiny ops per
    group (tiny DVE ops cost 2-8us each in-situ from fixed overhead +
    in-order queue blocking + SBUF-port contention with the Q7 gathers).
  * Layer-2 projection is interleaved into the layer-1 edge loop per block,
    so the layer-2 AllGather (half A) completes before layer-1 finishes.
  * Gathers run in a skewed pipeline: A-half gathers are issued SKEW pairs
    ahead of B-half ones, hiding the B-half AllGather latency.
  * ev/mt elementwise work is pair-granular (half the instruction count).
  * er table stored bf16 (no per-group cast).

One-hot window matrices (oh/ohT) come from the host as bf16; the gathered
row layout, dst-sorted edge schedule, and the skipped softmax
max-subtraction are as in v2.
"""

import math
import os
import sys
from contextlib import ExitStack

import numpy as np

for _p in ("/opt/trn_rl_repo", "/root/.axon_site/_ro/trn_rl_repo"):
    if os.path.isdir(_p) and _p not in sys.path:
        sys.path.append(_p)

import ml_dtypes

import concourse.bass as bass
import concourse.tile as tile
from concourse import bacc, mybir
from concourse.bass_utils import run_bass_kernel_spmd

BF16 = ml_dtypes.bfloat16

N = 50000
E = 800000
F_IN = 128
H, D, C = 4, 32, 47
HD = H * D            # 128
HC = H * C            # 188
NEG_SLOPE = 0.2

NCORES = 8
P = 128
NPC = N // NCORES         # 6250
G = math.ceil(NPC / P)    # 49
NPAD = G * P              # 6272
GA = 25                   # groups in half A
HSPLIT = GA * P           # 3200 rows per core in half A
HB = NPAD - HSPLIT        # 3072 rows per core in half B
NTA = HSPLIT * NCORES     # 25600 (< 2^15)
NTB = HB * NCORES         # 24576 (< 2^15)

W1 = HD + H               # 132
W2 = HC + H               # 192
ELEM = 256                # gathered row width in bf16 -> 512B
SKEW = 4                  # pairs of A-half gathers issued ahead

LAST_EXEC_NS = None


def _schedule(src, dst):
    """Edge schedule: per (core, group, half) slot runs + one-hot tensors."""
    order = np.argsort(dst, kind="stable")
    s_src = src[order].astype(np.int64)
    s_dst = dst[order].astype(np.int64)

    core_of = s_dst // NPC
    g_of = (s_dst % NPC) // P
    win = (s_dst % NPC) % P

    src_c = s_src // NPC               # owner core of src
    src_i = s_src % NPC                # within-core index (< 6250)
    half = (src_i >= HSPLIT).astype(np.int64)
    idx_val = np.where(half == 0, src_c * HSPLIT + src_i,
                       src_c * HB + (src_i - HSPLIT))

    cgh = (core_of * G + g_of) * 2 + half
    order2 = np.argsort(cgh, kind="stable")
    cgh = cgh[order2]
    idx_val = idx_val[order2]
    win = win[order2]

    counts = np.bincount(cgh, minlength=NCORES * G * 2).reshape(NCORES, G, 2)
    KA = int(math.ceil(counts[:, :, 0].max() / P))
    KB = int(math.ceil(counts[:, :, 1].max() / P))
    K = KA + KB

    starts = np.zeros(NCORES * G * 2 + 1, dtype=np.int64)
    np.cumsum(counts.ravel(), out=starts[1:])
    pos_in_run = np.arange(len(cgh)) - starts[cgh]

    base = np.where(cgh % 2 == 0, 0, KA * P)
    flat = base + pos_in_run
    cg = cgh // 2

    idx_flat = np.zeros((NCORES * G, K * P), dtype=np.int64)   # pad -> row 0
    dstl_flat = np.full((NCORES * G, K * P), -1, dtype=np.int64)  # pad -> -1
    idx_flat[cg, flat] = idx_val
    dstl_flat[cg, flat] = win

    idx_flat = idx_flat.reshape(NCORES, G, K, P)
    dstl_flat = dstl_flat.reshape(NCORES, G, K, P)

    def wrap(a):
        # [NC, G, n] slot-major -> [NC, G, 128, n/16] wrapped+replicated
        n = a.shape[-1]
        w = a.reshape(*a.shape[:-1], n // 16, 16)
        w = np.swapaxes(w, -1, -2)                    # [.., 16, n/16]
        return np.tile(w, (1, 1, 8, 1)).astype(np.int16)

    idxA_w = wrap(idx_flat[:, :, :KA, :].reshape(NCORES, G, KA * P))
    idxB_w = wrap(idx_flat[:, :, KA:, :].reshape(NCORES, G, KB * P))

    # one-hots, built per core to bound peak memory
    m_ar = np.arange(P, dtype=np.int64)
    oh = np.empty((NCORES, G, P, K * P), dtype=BF16)
    ohT = np.empty((NCORES, G, P, K * P), dtype=BF16)
    for c in range(NCORES):
        d = dstl_flat[c]                                   # [G, K, P]
        eq = (d[:, :, :, None] == m_ar).astype(BF16)       # [G, K, Pj, Pm]
        oh[c] = eq.transpose(0, 2, 1, 3).reshape(G, P, K * P)   # [j,(k,m)]
        ohT[c] = eq.transpose(0, 3, 1, 2).reshape(G, P, K * P)  # [m,(k,j)]

    return dict(idxA_w=idxA_w, idxB_w=idxB_w, oh=oh, ohT=ohT, KA=KA, KB=KB)


def _blockdiag(a, hd, h, dim):
    out = np.zeros((hd, h), dtype=np.float32)
    for i in range(h):
        out[i * dim:(i + 1) * dim, i] = a[i]
    return out


def _build_program(KA, KB):
    K = KA + KB
    nc = bacc.Bacc("TRN2", target_bir_lowering=False, debug=False,
                   num_devices=NCORES)
    dt = mybir.dt
    f32, bf16, i16 = dt.float32, dt.bfloat16, dt.int16
    AF = mybir.ActivationFunctionType

    def inp(name, shape, d=f32):
        return nc.dram_tensor(name, shape, d, kind="ExternalInput").ap()

    x_own = inp("x_own", [NPAD, F_IN])
    w1cat = inp("w1cat", [F_IN, W1 + H], bf16)
    w2cat = inp("w2cat", [F_IN, W2 + H], bf16)
    b1_t = inp("b1_t", [P, HD])
    b2m_t = inp("b2m_t", [P, C])
    ident_t = inp("ident_t", [P, P])
    idxA_in = inp("idxA_in", [G, P, KA * 8], i16)
    idxB_in = inp("idxB_in", [G, P, KB * 8], i16)
    oh_in = inp("oh_in", [G, P, K * P], bf16)
    ohT_in = inp("ohT_in", [G, P, K * P], bf16)

    y_out = nc.dram_tensor("y_out", [NPAD, C], f32, kind="ExternalOutput").ap()

    tab1_own = nc.dram_tensor("tab1_own", [NPAD, ELEM], bf16).ap()
    tab1A = nc.dram_tensor("tab1A", [NTA, ELEM], bf16, addr_space="Shared").ap()
    tab1B = nc.dram_tensor("tab1B", [NTB, ELEM], bf16, addr_space="Shared").ap()
    er1_d = nc.dram_tensor("er1_d", [NPAD, H], bf16).ap()
    tab2_own = nc.dram_tensor("tab2_own", [NPAD, ELEM], bf16).ap()
    tab2A = nc.dram_tensor("tab2A", [NTA, ELEM], bf16, addr_space="Shared").ap()
    tab2B = nc.dram_tensor("tab2B", [NTB, ELEM], bf16, addr_space="Shared").ap()
    er2_d = nc.dram_tensor("er2_d", [NPAD, H], bf16).ap()

    pairs = [(g, g + 1) if g + 1 < G else (g,) for g in range(0, G, 2)]
    # layer-1 epilogue blocks; a block boundary at 25 lets the layer-2
    # half-A AllGather start once groups 0..24 are projected
    blocks = [(0, 8), (8, 16), (16, 25), (25, 33), (33, 41), (41, 49)]

    with tile.TileContext(nc) as tc, ExitStack() as ctx:
        const = ctx.enter_context(tc.tile_pool(name="const", bufs=1))
        sb = ctx.enter_context(tc.tile_pool(name="sb", bufs=3))
        ohp = ctx.enter_context(tc.tile_pool(name="ohp", bufs=3))
        mtp = ctx.enter_context(tc.tile_pool(name="mtp", bufs=2))
        fin = ctx.enter_context(tc.tile_pool(name="fin", bufs=1))
        gatA = ctx.enter_context(tc.tile_pool(name="gatA", bufs=SKEW + 1))
        gatB = ctx.enter_context(tc.tile_pool(name="gatB", bufs=2))
        ps = ctx.enter_context(tc.tile_pool(name="ps", bufs=2, space="PSUM"))
        psg = ctx.enter_context(tc.tile_pool(name="psg", bufs=2, space="PSUM"))
        big = ctx.enter_context(tc.tile_pool(name="big", bufs=1))

        ident = const.tile([P, P], f32)
        nc.sync.dma_start(ident[:], ident_t[:])
        b1s = const.tile([P, HD], f32)
        nc.sync.dma_start(b1s[:], b1_t[:])
        b2ms = const.tile([P, C], f32)
        nc.sync.dma_start(b2ms[:], b2m_t[:])
        w1 = const.tile([P, W1 + H], bf16)
        nc.sync.dma_start(w1[:], w1cat[:])
        w2 = const.tile([P, W2 + H], bf16)
        nc.sync.dma_start(w2[:], w2cat[:])

        h1 = big.tile([P, G, F_IN], f32)
        stage = big.tile([P, G, W2], bf16)    # seg staging, both layers
        zsb = big.tile([P, G, C], f32)        # layer-2 z - zmax
        ssb = big.tile([P, G], f32)

        # ---------------- projection ----------------
        def project(src_tile_of, wcat, width, tab_own_d, er_d, glo, ghi):
            for g in range(glo, ghi):
                xt = src_tile_of(g)
                xT_ps = ps.tile([F_IN, P], f32, space="PSUM", tag="xT_ps")
                nc.tensor.transpose(xT_ps[:], xt[:], ident[:])
                xT = sb.tile([F_IN, P], bf16, tag="xT")
                nc.vector.tensor_copy(xT[:], xT_ps[:])
                pr = ps.tile([P, width + H], f32, space="PSUM", tag="proj")
                nc.tensor.matmul(pr[:], lhsT=xT[:], rhs=wcat[:, :width + H],
                                 start=True, stop=True)
                tb = sb.tile([P, width], bf16, tag="tabrow")
                nc.scalar.activation(tb[:], pr[:, :width], AF.Copy)
                nc.sync.dma_start(tab_own_d[g * P:(g + 1) * P, :width], tb[:])
                er = sb.tile([P, H], bf16, tag="errow")
                nc.scalar.activation(er[:], pr[:, width:width + H], AF.Copy)
                nc.sync.dma_start(er_d[g * P:(g + 1) * P, :], er[:])

        def x_tile(g):
            t = sb.tile([P, F_IN], f32, tag="xload")
            nc.sync.dma_start(t[:], x_own[g * P:(g + 1) * P, :])
            return t

        def h1_tile(g):
            return h1[:, g, :]

        def allgather(src_d, dst_d, lo, hi):
            nc.gpsimd.collective_compute(
                "AllGather", mybir.AluOpType.bypass,
                replica_groups=[list(range(NCORES))],
                ins=[src_d[lo:hi, :]], outs=[dst_d[:]])

        # ---------------- edge phase ----------------
        def edge_phase(tabA, tabB, er_d, width, stash_cb):
            pend = {}

            def issue(pi, hkey):
                if pi >= len(pairs):
                    return
                pair = pairs[pi]
                npair = len(pair)
                tab, Kh, idx_in = ((tabA, KA, idxA_in) if hkey == "A"
                                   else (tabB, KB, idxB_in))
                pool = gatA if hkey == "A" else gatB
                it = sb.tile([P, npair * Kh * 8], i16, tag=f"idx{hkey}")
                for gi, g in enumerate(pair):
                    nc.sync.dma_start(
                        it[:, gi * Kh * 8:(gi + 1) * Kh * 8], idx_in[g])
                gt = pool.tile([P, npair * Kh, ELEM], bf16, tag=f"gt{hkey}")
                nc.gpsimd.dma_gather(
                    out_ap=gt[:], in_ap=tab[:],
                    idxs_ap=it[:], num_idxs=npair * Kh * P,
                    num_idxs_reg=npair * Kh * P, elem_size=ELEM,
                    single_packet=False)
                pend[(pi, hkey)] = gt

            for pi in range(SKEW):
                issue(pi, "A")

            for pi, pair in enumerate(pairs):
                issue(pi, "B")
                issue(pi + SKEW, "A")
                gtA_t = pend.pop((pi, "A"))
                gtB_t = pend.pop((pi, "B"))
                npair = len(pair)

                oh_ts, ohT_ts, erw_ts = [], [], []
                for g in pair:
                    oh_t = ohp.tile([P, K, P], bf16, tag="oh")
                    nc.sync.dma_start(
                        oh_t[:].rearrange("p k m -> p (k m)"), oh_in[g])
                    ohT_t = ohp.tile([P, K, P], bf16, tag="ohT")
                    nc.sync.dma_start(
                        ohT_t[:].rearrange("p k m -> p (k m)"), ohT_in[g])
                    erw = sb.tile([P, H], bf16, tag="erw")
                    nc.sync.dma_start(erw[:], er_d[g * P:(g + 1) * P, :])
                    oh_ts.append(oh_t)
                    ohT_ts.append(ohT_t)
                    erw_ts.append(erw)

                mts = {}
                for hkey, Kh, coh, gt in (("A", KA, 0, gtA_t),
                                          ("B", KB, KA, gtB_t)):
                    nk = npair * Kh
                    erp = psg.tile([P, nk, H], f32, space="PSUM", tag="erp")
                    for gi in range(npair):
                        for c in range(Kh):
                            nc.tensor.matmul(
                                erp[:, gi * Kh + c, :],
                                lhsT=ohT_ts[gi][:, coh + c, :],
                                rhs=erw_ts[gi][:], start=True, stop=True)
                    ev = sb.tile([P, nk, H], f32, tag=f"ev{hkey}")
                    nc.vector.tensor_tensor(
                        out=ev[:], in0=gt[:, :, width - H:width],
                        in1=erp[:], op=mybir.AluOpType.add)
                    nc.vector.scalar_tensor_tensor(
                        out=ev[:], in0=ev[:], scalar=NEG_SLOPE, in1=ev[:],
                        op0=mybir.AluOpType.mult, op1=mybir.AluOpType.max)
                    mt = mtp.tile([P, nk, width], bf16, tag=f"mt{hkey}")
                    nc.scalar.activation(
                        mt[:, :, width - H:width], ev[:], AF.Exp)
                    nc.vector.tensor_tensor(
                        out=mt[:, :, :width - H].rearrange(
                            "p k (h d) -> p k h d", h=H),
                        in0=gt[:, :, :width - H].rearrange(
                            "p k (h d) -> p k h d", h=H),
                        in1=mt[:, :, width - H:width, None].to_broadcast(
                            [P, nk, H, (width - H) // H]),
                        op=mybir.AluOpType.mult)
                    mts[hkey] = mt

                for gi, g in enumerate(pair):
                    seg = psg.tile([P, width], f32, space="PSUM", tag="seg")
                    for bi, (hkey, Kh, coh) in enumerate(
                            (("A", KA, 0), ("B", KB, KA))):
                        mt = mts[hkey]
                        for c in range(Kh):
                            nc.tensor.matmul(
                                seg[:], lhsT=oh_ts[gi][:, coh + c, :],
                                rhs=mt[:, gi * Kh + c, :],
                                start=(bi == 0 and c == 0),
                                stop=(bi == 1 and c == Kh - 1))
                    stash_cb(g, seg)

        # ---------------- layer epilogues (batched) ----------------
        def stash1(g, seg):
            nc.scalar.activation(stage[:, g, :W1], seg[:, :W1], AF.Copy)
            for (g0, g1) in blocks:
                if g == g1 - 1:
                    l1_finale(g0, g1)

        def l1_finale(g0, g1):
            nb = g1 - g0
            V = stage[:, g0:g1, :]
            dn = sb.tile([P, nb, H], f32, tag="dn")
            nc.vector.tensor_scalar_max(dn[:], V[:, :, HD:HD + H], 1e-30)
            rd = sb.tile([P, nb, H], f32, tag="rd")
            nc.vector.reciprocal(rd[:], dn[:])
            ht = fin.tile([P, nb, F_IN], f32, tag="ht")
            nc.vector.tensor_tensor(
                out=ht[:].rearrange("p g (h d) -> p g h d", h=H),
                in0=V[:, :, :HD].rearrange("p g (h d) -> p g h d", h=H),
                in1=rd[:, :, :, None].to_broadcast([P, nb, H, D]),
                op=mybir.AluOpType.mult)
            nc.vector.tensor_tensor(
                out=ht[:], in0=ht[:],
                in1=b1s[:, None, :].to_broadcast([P, nb, HD]),
                op=mybir.AluOpType.add)
            mn = fin.tile([P, nb, F_IN], f32, tag="mn")
            nc.vector.tensor_scalar_min(mn[:], ht[:], 0.0)
            nc.scalar.activation(mn[:], mn[:], AF.Exp)
            nc.vector.scalar_tensor_tensor(
                out=h1[:, g0:g1, :], in0=mn[:], scalar=-1.0, in1=ht[:],
                op0=mybir.AluOpType.add, op1=mybir.AluOpType.max)
            # layer-2 projection for the completed block
            project(h1_tile, w2, W2, tab2_own, er2_d, g0, g1)
            if g1 == GA:
                allgather(tab2_own, tab2A, 0, HSPLIT)
            if g1 == G:
                allgather(tab2_own, tab2B, HSPLIT, NPAD)

        def stash2(g, seg):
            nc.scalar.activation(stage[:, g, :W2], seg[:, :W2], AF.Copy)

        def l2_finale():
            dn = fin.tile([P, G, H], f32, tag="dn2")
            nc.vector.tensor_scalar_max(dn[:], stage[:, :, HC:HC + H], 1e-30)
            rd = fin.tile([P, G, H], f32, tag="rd2")
            nc.vector.reciprocal(rd[:], dn[:])
            nc.vector.tensor_scalar_mul(rd[:], rd[:], 1.0 / H)
            nc.vector.tensor_tensor(
                out=stage[:, :, :HC].rearrange("p g (h c) -> p g h c", h=H),
                in0=stage[:, :, :HC].rearrange("p g (h c) -> p g h c", h=H),
                in1=rd[:, :, :, None].to_broadcast([P, G, H, C]),
                op=mybir.AluOpType.mult)
            nc.vector.reduce_sum(
                zsb[:], stage[:, :, :HC].rearrange("p g (h c) -> p g c h", h=H),
                axis=mybir.AxisListType.X)
            nc.vector.tensor_tensor(
                out=zsb[:], in0=zsb[:],
                in1=b2ms[:, None, :].to_broadcast([P, G, C]),
                op=mybir.AluOpType.add)
            zm = fin.tile([P, G], f32, tag="zm")
            nc.vector.reduce_max(zm[:], zsb[:], axis=mybir.AxisListType.X)
            nc.vector.tensor_tensor(
                out=zsb[:], in0=zsb[:],
                in1=zm[:, :, None].to_broadcast([P, G, C]),
                op=mybir.AluOpType.subtract)
            for g in range(G):
                es = sb.tile([P, C], f32, tag="es")
                nc.scalar.activation(es[:], zsb[:, g, :], AF.Exp,
                                     accum_out=ssb[:, g:g + 1])
            lg = fin.tile([P, G], f32, tag="lg")
            nc.scalar.activation(lg[:], ssb[:], AF.Ln)
            yt = fin.tile([P, G, C], f32, tag="yt")
            nc.vector.tensor_tensor(
                out=yt[:], in0=zsb[:],
                in1=lg[:, :, None].to_broadcast([P, G, C]),
                op=mybir.AluOpType.subtract)
            nc.sync.dma_start(
                y_out[:].rearrange("(g p) c -> p g c", p=P), yt[:])

        # ---------------- run the two layers ----------------
        project(x_tile, w1, W1, tab1_own, er1_d, 0, GA)
        allgather(tab1_own, tab1A, 0, HSPLIT)
        project(x_tile, w1, W1, tab1_own, er1_d, GA, G)
        allgather(tab1_own, tab1B, HSPLIT, NPAD)
        edge_phase(tab1A, tab1B, er1_d, W1, stash1)
        edge_phase(tab2A, tab2B, er2_d, W2, stash2)
        l2_finale()

    nc.compile()
    return nc


def kernel(x, src, dst, W1s, W1d, al1, ar1, b1, W2s, W2d, al2, ar2, b2):
    global LAST_EXEC_NS
    x = np.asarray(x, dtype=np.float32)
    src = np.asarray(src, dtype=np.int32)
    dst = np.asarray(dst, dtype=np.int32)

    sch = _schedule(src, dst)
    KA, KB = sch["KA"], sch["KB"]

    def wcat(Ws, Wd, al, ar, hd, h, dim):
        Ws = np.asarray(Ws, np.float32)
        Wd = np.asarray(Wd, np.float32)
        wel = Ws @ _blockdiag(np.asarray(al, np.float32), hd, h, dim)
        wer = Wd @ _blockdiag(np.asarray(ar, np.float32), hd, h, dim)
        return np.concatenate([Ws, wel, wer], axis=1)

    w1c = wcat(W1s, W1d, al1, ar1, HD, H, D).astype(BF16)
    w2c = wcat(W2s, W2d, al2, ar2, HC, H, C).astype(BF16)

    ident_np = np.eye(P, dtype=np.float32)
    b1_np = np.tile(np.asarray(b1, np.float32)[None, :], (P, 1))
    b2m_np = np.tile(np.asarray(b2, np.float32).reshape(H, C).mean(0)[None, :],
                     (P, 1))

    x_pad = np.zeros((NCORES, NPAD, F_IN), np.float32)
    x_pad[:, :NPC, :] = x.reshape(NCORES, NPC, F_IN)

    nc = _build_program(KA, KB)

    in_maps = []
    for c in range(NCORES):
        in_maps.append({
            "x_own": x_pad[c],
            "w1cat": w1c, "w2cat": w2c,
            "b1_t": b1_np, "b2m_t": b2m_np,
            "ident_t": ident_np,
            "idxA_in": sch["idxA_w"][c], "idxB_in": sch["idxB_w"][c],
            "oh_in": sch["oh"][c], "ohT_in": sch["ohT"][c],
        })

    res = run_bass_kernel_spmd(nc, in_maps, list(range(NCORES)),
                               trace=bool(os.environ.get("GAT_TRACE")))
    LAST_EXEC_NS = res.exec_time_ns
    out = np.concatenate(
        [res.results[c]["y_out"][:NPC] for c in range(NCORES)], axis=0)
    return out.astype(np.float32)
